# revision 27
# baseline (speedup 1.0000x reference)
"""Trainium2 Bass kernel for causal multi-head attention with RoPE.

Full module: qkv = x @ w_qkv; RoPE(q, k); causal softmax attention;
out = attn_out @ w_out.  x: [2, 2048, 1024], 16 heads x 64 dim.

Sharding: 8 cores = 2 batches x 4 head-groups (4 heads/core).  Each core
computes its batch's q/k/v for its heads, runs attention, and produces a
partial [2048, 1024] output through its slice of w_out.  Host sums the 4
partials per batch.
"""

import os
import sys

import numpy as np

for _p in ("/opt/trn_rl_repo", "/root/.axon_site/_ro/trn_rl_repo"):
    if os.path.isdir(_p) and _p not in sys.path:
        sys.path.append(_p)

import concourse.bass as bass
import concourse.mybir as mybir
import concourse.tile as tile
from concourse.masks import make_upper_triangular

F32 = mybir.dt.float32

# Problem constants (hardcoded per contest rules)
B = 2
N = 2048
D = 1024
HEADS = 16
DH = 64
N_CORES = 8
HL = HEADS // (N_CORES // B)  # heads per core = 4


F32R = mybir.dt.float32r


def build_attention_nc(
    n=N,
    d=D,
    hl=HL,
    dh=DH,
    mm_dt_qkv=F32R,
    mm_dt_attn=F32R,
    mm_dt_out=F32R,
    qt=512,
    cap_waits=True,
):
    """Build the per-core Bass module.  All cores run the same program (SPMD)."""
    nc = bass.Bass("TRN2", target_bir_lowering=False, debug=False)

    KC = d // 128          # contraction chunks for qkv production
    CT = (hl * dh) // 128  # column tiles for q/k (2 heads per tile)
    NQT = n // qt          # query tiles
    KPQ = qt // 128        # key chunks per query tile
    NSB = n // 128         # seq blocks / key chunks
    OC = (hl * dh) // 128  # w_out contraction chunks from this core
    assert OC == CT
    NOT = d // 512         # output N tiles
    scale = float(dh) ** -0.5

    xT = nc.dram_tensor("xT", [d, n], F32, kind="ExternalInput").ap()
    wq = nc.dram_tensor("wq", [d, hl * dh], F32, kind="ExternalInput").ap()
    wk = nc.dram_tensor("wk", [d, hl * dh], F32, kind="ExternalInput").ap()
    wv = nc.dram_tensor("wv", [d, hl * dh], F32, kind="ExternalInput").ap()
    wo = nc.dram_tensor("wo", [hl * dh, d], F32, kind="ExternalInput").ap()
    cosT = nc.dram_tensor("cosT", [128, n], F32, kind="ExternalInput").ap()
    sinT = nc.dram_tensor("sinT", [128, n], F32, kind="ExternalInput").ap()
    outp = nc.dram_tensor("out_partial", [n, d], F32, kind="ExternalOutput").ap()

    with tile.TileContext(nc) as tc:
        with tc.tile_pool(name="pers", bufs=1) as pers:
            # Persistent SBUF tensors alive from production through attention
            qT_sb = [pers.tile([128, n], mm_dt_attn, tag=f"qT{i}", name=f"qT{i}") for i in range(CT)]
            kT_sb = [pers.tile([128, n], mm_dt_attn, tag=f"kT{i}", name=f"kT{i}") for i in range(CT)]
            # v in natural layout, ones column appended per head
            v_sb = [pers.tile([128, hl, dh + 1], mm_dt_attn, tag=f"v{i}", name=f"v{i}") for i in range(NSB)]
            onec_sb = pers.tile([128, 1], F32, tag="onec", name="onec")
            tri_sb = pers.tile([128, 128], F32, tag="tri", name="tri")

            nc.vector.memset(onec_sb, 1.0)
            make_upper_triangular(nc, tri_sb[:], val=1.0, diag=True)

            # ---- Stages B/C: produce qT, kT (RoPE'd) and v (+ones) ----
            with tc.tile_pool(name="stg", bufs=1) as stg:
                x_sb = [stg.tile([128, n], mm_dt_qkv, tag=f"x{kc}", name=f"x{kc}") for kc in range(KC)]
                wq_sb = stg.tile([128, KC, hl * dh], mm_dt_qkv, tag="wq", name="wq")
                wk_sb = stg.tile([128, KC, hl * dh], mm_dt_qkv, tag="wk", name="wk")
                wv_sb = stg.tile([128, KC, hl * dh], mm_dt_qkv, tag="wv", name="wv")
                cos_sb = stg.tile([128, n], F32, tag="cos", name="cos")
                sin_sb = stg.tile([128, n], F32, tag="sin", name="sin")

                # PE warm-up during the initial DMA loads: dummy matmuls on
                # a memset tile keep HAM at full clock before qkv starts.
                warm = stg.tile([128, 512], mm_dt_qkv, tag="warm", name="warm")
                warmf = stg.tile([128, 512], F32, tag="warmf", name="warmf")
                nc.vector.memset(warmf, 0.0)
                nc.vector.tensor_copy(warm, warmf)
                with tc.tile_pool(name="pswarm", bufs=1, space="PSUM") as pswarm:
                    wps = pswarm.tile([128, 512], F32, tag="wps", name="wps")
                    for i in range(72):
                        nc.tensor.matmul(
                            wps, warm[:, 0:128], warm, start=(i == 0), stop=(i == 71)
                        )

                nc.sync.dma_start(cos_sb, cosT)
                nc.sync.dma_start(sin_sb, sinT)
                if mm_dt_qkv != F32:
                    # f32r operands must be rounded by a compute producer:
                    # stage loads through fp32 tmp tiles, cast on DVE.
                    with tc.tile_pool(name="ldtmp", bufs=3) as ldtmp:
                        for w_dram, w_tile in ((wq, wq_sb), (wk, wk_sb), (wv, wv_sb)):
                            wtmp = ldtmp.tile([128, KC, hl * dh], F32, tag="wtmp", name="wtmp")
                            nc.sync.dma_start(
                                wtmp, w_dram.rearrange("(kc p) m -> p kc m", p=128)
                            )
                            nc.scalar.activation(w_tile, wtmp, mybir.ActivationFunctionType.Copy)
                        for kc in range(KC):
                            xtmp = ldtmp.tile([128, n], F32, tag="xtmp", name="xtmp")
                            nc.sync.dma_start(xtmp, xT[kc * 128 : (kc + 1) * 128, :])
                            nc.scalar.activation(x_sb[kc], xtmp, mybir.ActivationFunctionType.Copy)
                else:
                    nc.sync.dma_start(wq_sb, wq.rearrange("(kc p) m -> p kc m", p=128))
                    nc.sync.dma_start(wk_sb, wk.rearrange("(kc p) m -> p kc m", p=128))
                    nc.sync.dma_start(wv_sb, wv.rearrange("(kc p) m -> p kc m", p=128))
                    for kc in range(KC):
                        nc.sync.dma_start(x_sb[kc], xT[kc * 128 : (kc + 1) * 128, :])
                x_mm, wq_mm, wk_mm, wv_mm = x_sb, wq_sb, wk_sb, wv_sb

                # qT / kT with fused RoPE, interleaved with v production so
                # the PE fills RoPE-paced gaps with v matmuls.
                with (
                    tc.tile_pool(name="psB", bufs=1, space="PSUM") as psB,
                    tc.tile_pool(name="psC", bufs=3, space="PSUM") as psC,
                    tc.tile_pool(name="ropet", bufs=4) as ropet,
                ):
                    def emit_v_block(sb):
                        psv = psC.tile([128, hl * dh], F32, tag="v", name="v")
                        for kc in range(KC):
                            nc.tensor.matmul(
                                psv,
                                x_mm[kc][:, sb * 128 : (sb + 1) * 128],
                                wv_mm[:, kc, :],
                                start=(kc == 0),
                                stop=(kc == KC - 1),
                            )
                        nc.scalar.activation(
                            v_sb[sb][:, :, 0:dh],
                            psv.rearrange("p (h e) -> p h e", h=hl),
                            mybir.ActivationFunctionType.Copy,
                        )
                        nc.vector.tensor_copy(
                            v_sb[sb][:, :, dh : dh + 1],
                            onec_sb[:, None, :].to_broadcast([128, hl, 1]),
                        )

                    gi = 0
                    v_emitted = [0]
                    for ct in range(CT):
                        for w_mm, dst in ((wq_mm, qT_sb), (wk_mm, kT_sb)):
                            ps = [
                                psB.tile([128, qt], F32, tag=f"b{st}", name=f"b{st}")
                                for st in range(NQT)
                            ]
                            for kc in range(KC):
                                for st in range(NQT):
                                    nc.tensor.matmul(
                                        ps[st],
                                        w_mm[:, kc, ct * 128 : (ct + 1) * 128],
                                        x_mm[kc][:, st * qt : (st + 1) * qt],
                                        start=(kc == 0),
                                        stop=(kc == KC - 1),
                                    )
                            for st in range(NQT):
                                sl = slice(st * qt, (st + 1) * qt)
                                raw_t = ropet.tile([128, qt], F32, tag="raw", name="raw")
                                a_t = ropet.tile([128, qt], F32, tag="a", name="a")
                                sh_t = ropet.tile([128, qt], F32, tag="sh", name="sh")
                                nc.scalar.activation(raw_t, ps[st], mybir.ActivationFunctionType.Copy)
                                # rotate_half as a partition swap (sign in sinT)
                                for hb in range(2):
                                    o = hb * 64
                                    nc.sync.dma_start(
                                        sh_t[o : o + 32, :], raw_t[o + 32 : o + 64, :]
                                    )
                                    nc.sync.dma_start(
                                        sh_t[o + 32 : o + 64, :], raw_t[o : o + 32, :]
                                    )
                                nc.vector.tensor_tensor(
                                    a_t, raw_t, cos_sb[:, sl], mybir.AluOpType.mult
                                )
                                nc.gpsimd.tensor_tensor(
                                    sh_t, sh_t, sin_sb[:, sl], mybir.AluOpType.mult
                                )
                                nc.vector.tensor_tensor(
                                    dst[ct][:, sl], a_t, sh_t, mybir.AluOpType.add
                                )
                                # weave v blocks proportionally into RoPE
                                gi += 1
                                target = gi * NSB // (CT * 2 * NQT)
                                while v_emitted[0] < target:
                                    emit_v_block(v_emitted[0])
                                    v_emitted[0] += 1
                    while v_emitted[0] < NSB:
                        emit_v_block(v_emitted[0])
                        v_emitted[0] += 1

            # ---- Stages D/E/F woven per query tile ----
            pers2 = tc.alloc_tile_pool(name="pers2", bufs=1)
            u_sb = [
                [pers2.tile([dh + 1, qt], F32, tag=f"u{h}_{t}", name=f"u{h}_{t}") for t in range(NQT)]
                for h in range(hl)
            ]
            outT_sb = [pers2.tile([128, n], mm_dt_out, tag=f"oT{i}", name=f"oT{i}") for i in range(CT)]
            wo_sb = pers2.tile([128, OC, d], mm_dt_out, tag="wo", name="wo")
            if mm_dt_out != F32:
                with tc.tile_pool(name="wotp", bufs=1) as wotp:
                    wotmp = wotp.tile([128, OC, d], F32, tag="wotmp", name="wotmp")
                    nc.sync.dma_start(wotmp, wo.rearrange("(kc p) m -> p kc m", p=128))
                    nc.vector.tensor_copy(wo_sb, wotmp)
            else:
                nc.sync.dma_start(wo_sb, wo.rearrange("(kc p) m -> p kc m", p=128))

            qT_mm, kT_mm, v_mm = qT_sb, kT_sb, v_sb
            oT_mm, wo_mm = outT_sb, wo_sb
            NHP = hl // 2  # head pairs (one q/k column tile each)
            with (
                tc.tile_pool(name="psS", bufs=2, space="PSUM") as psS,
                tc.tile_pool(name="psAV", bufs=1, space="PSUM") as psAV,
                tc.tile_pool(name="expp", bufs=12) as expp,
                tc.tile_pool(name="dramE", bufs=1, space="DRAM") as dramE,
                tc.tile_pool(name="bcp", bufs=6) as bcp,
                tc.tile_pool(name="fo", bufs=6) as fo,
            ):
                scr3 = dramE.tile([hl * NQT, qt], F32, tag="scr3", name="scr3")
                LAG = 5  # chunks of scores-ahead before the matching AV
                for t in range(NQT):
                    qsl = slice(t * qt, (t + 1) * qt)
                    pav = [psAV.tile([dh + 1, qt], F32, tag=f"av{h}", name=f"av{h}") for h in range(hl)]
                    ncc = KPQ * (t + 1)
                    e_ts = {}

                    def emit_scores(c, t=t, qsl=qsl, e_ts=e_ts):
                        j = c - KPQ * t
                        lo = max(0, j * 128)
                        for hp in range(NHP):
                            pss = psS.tile([128, 2, qt], F32, tag="s", name="s")
                            for g in range(2):
                                bp = 64 * g
                                nc.tensor.matmul(
                                    pss[:, g, :],
                                    kT_mm[hp][bp : bp + dh, c * 128 : (c + 1) * 128],
                                    qT_mm[hp][bp : bp + dh, qsl],
                                    start=True,
                                    stop=True,
                                )
                            e_t = expp.tile([128, 2, qt], mm_dt_attn, tag="e", name="e")
                            nc.scalar.activation(
                                e_t, pss, mybir.ActivationFunctionType.Exp, scale=scale
                            )
                            if j >= 0:
                                nc.gpsimd.tensor_tensor(
                                    e_t[:, :, lo : lo + 128],
                                    e_t[:, :, lo : lo + 128],
                                    tri_sb[:, None, :].to_broadcast([128, 2, 128]),
                                    mybir.AluOpType.mult,
                                )
                            e_ts[(c, hp)] = e_t

                    def emit_av(c, t=t, e_ts=e_ts, pav=pav, ncc=ncc):
                        lo = max(0, (c - KPQ * t) * 128)
                        for hp in range(NHP):
                            e_t = e_ts.pop((c, hp))
                            for g in range(2):
                                h = 2 * hp + g
                                nc.tensor.matmul(
                                    pav[h][:, lo:qt],
                                    v_mm[c][:, h, :],
                                    e_t[:, g, lo:qt],
                                    start=(c == 0),
                                    stop=(c == ncc - 1),
                                )

                    for c in range(ncc):
                        emit_scores(c)
                        if c >= LAG:
                            emit_av(c - LAG)
                    for c in range(max(0, ncc - LAG), ncc):
                        emit_av(c)
                    for h in range(hl):
                        nc.vector.tensor_copy(u_sb[h][t], pav[h])

                    # ---- E(t): reciprocal of this qtile's rowsums ----
                    rs4 = bcp.tile([hl, qt], F32, tag="rs4", name="rs4")
                    for h in range(hl):
                        nc.sync.dma_start(
                            rs4[h : h + 1, :], u_sb[h][t][dh : dh + 1, :]
                        )
                    nc.vector.reciprocal(rs4, rs4)
                    for h in range(hl):
                        i = h * NQT + t
                        nc.sync.dma_start(scr3[i : i + 1, :], rs4[h : h + 1, :])
                    for h in range(hl):
                        ct, bp = h // 2, 64 * (h % 2)
                        i = h * NQT + t
                        bc_t = bcp.tile([dh, qt], F32, tag="bc", name="bc")
                        nc.sync.dma_start(
                            bc_t, scr3[i : i + 1, :].to_broadcast([dh, qt])
                        )
                        nc.vector.tensor_tensor(
                            outT_sb[ct][bp : bp + dh, t * qt : (t + 1) * qt],
                            u_sb[h][t][0:dh, :],
                            bc_t,
                            mybir.AluOpType.mult,
                        )

                    # ---- F(t): output projection for this qtile's rows ----
                    for sb in range(t * qt // 128, (t + 1) * qt // 128):
                        for nt in range(NOT):
                            pso = psAV.tile(
                                [128, 512], F32,
                                tag=f"av{(sb % 2) * 2 + nt}", name="pso",
                            )
                            for kc in range(OC):
                                nc.tensor.matmul(
                                    pso,
                                    oT_mm[kc][:, sb * 128 : (sb + 1) * 128],
                                    wo_mm[:, kc, nt * 512 : (nt + 1) * 512],
                                    start=(kc == 0),
                                    stop=(kc == OC - 1),
                                )
                            o_t = fo.tile([128, 512], F32, tag="ot", name="ot")
                            if (sb + nt) % 2 == 0:
                                nc.vector.tensor_copy(o_t, pso)
                            else:
                                nc.scalar.activation(
                                    o_t, pso, mybir.ActivationFunctionType.Copy
                                )
                            nc.sync.dma_start(
                                outp[
                                    sb * 128 : (sb + 1) * 128, nt * 512 : (nt + 1) * 512
                                ],
                                o_t,
                            )
            pers2.release()
    if cap_waits:
        _cap_matmul_waits(nc)
    return nc


_CAPPED_INSTS = {
    "InstMatmult",
    "InstTensorTensor",
    "InstTensorCopy",
    "InstActivation",
    "InstTensorScalarAffineSelect",
    "InstTensorScalar",
    "InstTensorReduce",
    "InstMemset",
    "InstReciprocal",
    "InstLdweights",
    "InstTensorTensorScan",
    "InstIota",
    "InstDMACopy",
    "InstDrain",
}


def _cap_matmul_waits(nc, max_keep=1):
    """Walrus codegen allows only one sync-wait per compute instruction
    (S3 struct wait slots).  Move excess waits onto NoOps inserted just
    before, on the same engine; engines execute in order so the semantics
    are identical."""
    nop_id = 0
    for f in nc.m.functions:
        for blk in f.blocks:
            insts = blk.instructions
            idx = 0
            while idx < len(insts):
                inst = insts[idx]
                if (
                    type(inst).__name__ in _CAPPED_INSTS
                    and inst.sync_info is not None
                    and len(inst.sync_info.on_wait or []) > max_keep
                ):
                    waits = list(inst.sync_info.on_wait)
                    extra, keep = waits[:-max_keep], waits[-max_keep:]
                    inst.sync_info = mybir.SyncInfo(
                        on_wait=keep, on_update=list(inst.sync_info.on_update or [])
                    )
                    for w in extra:
                        nop = mybir.InstNoOp(name=f"I-mmwait-nop-{nop_id}")
                        nop_id += 1
                        nop.engine = inst.engine
                        nop.sync_info = mybir.SyncInfo(on_wait=[w], on_update=[])
                        insts.insert(idx, nop)
                        idx += 1
                idx += 1


def _rope_tables(n, dh, hl):
    """Host-side RoPE tables in transposed, 2-head-stacked, sign-folded form."""
    inv_freq = 1.0 / (10000.0 ** (np.arange(0, dh, 2, dtype=np.float32) / dh))
    t = np.arange(n, dtype=np.float32)
    freqs = np.outer(t, inv_freq).astype(np.float32)  # [n, dh/2]
    emb = np.concatenate([freqs, freqs], axis=-1)  # [n, dh]
    cos = np.cos(emb).astype(np.float32).T  # [dh, n]
    sin = np.sin(emb).astype(np.float32).T
    sin_signed = sin.copy()
    sin_signed[: dh // 2] *= -1.0
    cosT = np.ascontiguousarray(np.tile(cos, (128 // dh, 1)))
    sinT = np.ascontiguousarray(np.tile(sin_signed, (128 // dh, 1)))
    return cosT, sinT


_NC_CACHE = {}


def kernel(x, w_qkv, w_out):
    return run(x, w_qkv, w_out)[0]


def run(x, w_qkv, w_out, trace=False, build_kwargs=None):
    from concourse.bass_utils import run_bass_kernel_spmd

    x = np.asarray(x, dtype=np.float32)
    w_qkv = np.asarray(w_qkv, dtype=np.float32)
    w_out = np.asarray(w_out, dtype=np.float32)

    cosT, sinT = _rope_tables(N, DH, HL)
    in_maps = []
    for core in range(N_CORES):
        b = core // (N_CORES // B)
        g = core % (N_CORES // B)
        cs = slice(g * HL * DH, (g + 1) * HL * DH)
        in_maps.append(
            {
                "xT": np.ascontiguousarray(x[b].T),
                "wq": np.ascontiguousarray(w_qkv[:, cs]),
                "wk": np.ascontiguousarray(w_qkv[:, D:][:, cs]),
                "wv": np.ascontiguousarray(w_qkv[:, 2 * D :][:, cs]),
                "wo": np.ascontiguousarray(w_out[cs, :]),
                "cosT": cosT,
                "sinT": sinT,
            }
        )

    key = repr(sorted((build_kwargs or {}).items()))
    if key not in _NC_CACHE:
        _NC_CACHE[key] = build_attention_nc(**(build_kwargs or {}))
    nc = _NC_CACHE[key]

    res = run_bass_kernel_spmd(
        nc, in_maps, core_ids=list(range(N_CORES)), trace=trace
    )
    out = np.zeros((B, N, D), dtype=np.float32)
    for core in range(N_CORES):
        out[core // (N_CORES // B)] += res.results[core]["out_partial"]
    return out, res


if __name__ == "__main__":
    rng = np.random.default_rng(0)
    x = rng.standard_normal((B, N, D), dtype=np.float32)
    w_qkv = rng.standard_normal((D, 3 * D), dtype=np.float32) * D**-0.5
    w_out = rng.standard_normal((D, D), dtype=np.float32) * D**-0.5
    out = kernel(x, w_qkv, w_out)
    print("out", out.shape, out.dtype, float(np.abs(out).max()))
